# revision 17
# baseline (speedup 1.0000x reference)
"""Distributed Trainium2 kernel for nn_Attn_77970836292156.

Cross-attention block: fused QKV projection + per-head RMSNorm + RoPE +
bf16 SDPA (4096 keys = 2048 self + 2048 cross) + output projection.

Sharding: tensor-parallel on heads. 16 heads / 8 cores = 2 heads per core.
W_qkv / W_ckv column-sharded by head; every core holds full x, y (transposed,
bf16). Attention runs fully local per core in a transposed layout
(head-dims on partitions, positions on the free axis). An AllToAll converts
head-sharding -> sequence-sharding, then each core applies the full W_out to
its position slice (row-sharded matmul accumulated over all 1024 dims).

Structure (v3 -- rebuilt from trace analysis):
- Attention runs in FOUR 512-query chunks, each with its own (128KB)
  AllToAll, so collectives pipeline under later chunks' compute and only
  the last chunk's A2A is exposed (measured ~5-7us once cores are in
  lockstep, vs ~25us for the old single 256KB A2A at the end).
- PSUM: pv accumulators [128,512] x2 heads double-buffered (4 banks) +
  QK score tiles (2 banks) + out-proj accumulators (2 banks) = 8 banks,
  so consecutive chunks never stall on each other's normalize.
- Every matmul has a 128-column stationary operand => fast-weight-load
  stays enabled (a 65-wide PV stationary disables FWL and costs
  ~100ns/matmul). The PV stationary per head is [v(64) | ones(64)] (or
  mirrored), so the PSUM rows opposite the o-rows accumulate 64 copies
  of the softmax denominator: the reciprocal (ACT Ln + Exp(-x), ordered
  Ln/Ln then Exp/Exp = 2 table loads) runs on 64 partitions directly and
  needs only a partition-shift DMA, no broadcast primitive.
- Softmax exp split across engines per head: h0 on ACT (table exp), h1 on
  DVE via a Schraudolph bit-trick (bits = trunc(score*a + b) as int16,
  reinterpreted bf16).
- RMSNorm rsqrt batched q+k (one Ln + one Exp over [2,4096] as soon as
  the x-side mean-squares land, during the y projections) + ck at
  phase-1 end: 4 ACT table loads total in phase 1.
- RoPE all on DVE (GpSimd tensor ops measured 1.4us/op -- useless).
  K is roped in two FULL-WIDTH [128,2048] passes (per-element DVE cost
  ~5x lower than 512-wide slices); q is roped per 512-chunk: q0 in the
  phase-1 foreground, q1/q2/q3 interleaved one-op-per-kc into attention
  chunks 0/1/2 so they never delay the chunk that needs them.
- A2A staging is ONE transposed-AP DMA per chunk; out-proj loads are one
  DMA per chunk emitted AFTER the last collective so the in-order SP
  queue can never wedge on a collective wait. Both out-projections are
  emitted after chunk 3: pair 0's matmuls execute inside the final A2A's
  flight window, so the exposed tail is ~(A2A + pair-1 matmuls) only.
- x/y are consumed through a 3-deep chunk pool with the x0 load issued
  FIRST (before the small weight loads): the first projection starts
  ~7us in instead of ~20us.
"""

import os

import numpy as np
import ml_dtypes

import concourse.bass as bass
import concourse.tile as tile
from concourse import bacc, mybir
from concourse.bass_utils import run_bass_kernel_spmd

BF16 = mybir.dt.bfloat16
F32 = mybir.dt.float32
I16 = mybir.dt.int16
I32 = mybir.dt.int32
AF = mybir.ActivationFunctionType
RMAGIC = 0x7EF127EA  # f32 reciprocal Newton seed: x0_bits = RMAGIC - d_bits

# Problem constants (hardcoded per spec).
N = 2048        # query positions
M = 2048        # cross positions
NK = N + M      # total keys
D = 1024        # model dim
H = 16          # heads
DH = 64         # head dim
HL = 2          # heads per core
DL = HL * DH    # local head dims = 128
P = 128
NCORES = 8
EPS = 1e-6
ROPE_BASE = 10000.0
SCALE = 0.125   # 1/sqrt(64)
KC = NK // P    # 32 key chunks of 128
NCH = 4         # query chunks
CQ = N // NCH   # 512 queries per chunk
CPC = CQ // NCORES  # 64 positions per core per chunk

# Schraudolph exp constants for bf16 bits = trunc(score*EXA + EXB):
#   bits = 128*(score*SCALE*log2 e) + 127*128 - 5.5 (minimax centering)
#   + 0.5 (truncation compensation)
EXA = SCALE * 128.0 * 1.4426950408889634
EXB = 16251.0

LAST_RESULT = None  # test harness reads exec_time_ns from here


def build_nc():
    nc = bacc.Bacc()

    # ---------------- DRAM parameters ----------------
    # x/y arrive host-prearranged chunk-major [p, chunk, f, 512] so each
    # position-chunk load is one contiguous 8KB run per partition.
    xT = nc.declare_dram_parameter("xT", [P, 4, 8, 512], BF16, isOutput=False)
    yT = nc.declare_dram_parameter("yT", [P, 4, 8, 512], BF16, isOutput=False)
    wq = nc.declare_dram_parameter("wq", [P, 8, DL], BF16, isOutput=False)
    wk = nc.declare_dram_parameter("wk", [P, 8, DL], BF16, isOutput=False)
    wv = nc.declare_dram_parameter("wv", [P, 8, DL], BF16, isOutput=False)
    wck = nc.declare_dram_parameter("wck", [P, 8, DL], BF16, isOutput=False)
    wcv = nc.declare_dram_parameter("wcv", [P, 8, DL], BF16, isOutput=False)
    wo = nc.declare_dram_parameter("wo", [P, 8, D], BF16, isOutput=False)
    bo = nc.declare_dram_parameter("bo", [1, D], BF16, isOutput=False)
    cq = nc.declare_dram_parameter("cq", [P, N], BF16, isOutput=False)
    sq = nc.declare_dram_parameter("sq", [P, N], BF16, isOutput=False)
    ckc = nc.declare_dram_parameter("ckc", [P, NK], BF16, isOutput=False)
    cks = nc.declare_dram_parameter("cks", [P, NK], BF16, isOutput=False)
    hmask = nc.declare_dram_parameter("hmask", [P, HL], BF16, isOutput=False)
    hsel = nc.declare_dram_parameter("hsel", [HL, P], BF16, isOutput=False)
    ident = nc.declare_dram_parameter("ident", [P, P], BF16, isOutput=False)
    rotm = nc.declare_dram_parameter("rotm", [P, P], BF16, isOutput=False)
    out_ext = nc.declare_dram_parameter("out", [2 * P, D], F32, isOutput=True)

    # A2A bounce buffers, one pair per q chunk (collectives can't touch I/O
    # tensors; separate tensors keep chunk deps independent).
    a2a_in = [nc.dram_tensor(f"a2a_in{c}", [NCORES, P, CPC], BF16)
              for c in range(NCH)]
    a2a_out = [nc.dram_tensor(f"a2a_out{c}", [NCORES, P, CPC], BF16)
               for c in range(NCH)]

    with tile.TileContext(nc) as tc, \
            tc.tile_pool(name="singles", bufs=1) as singles, \
            tc.tile_pool(name="qrope", bufs=2) as qrope:

        # ---------------- static SBUF loads (x chunk 0 issued first) -------
        xt0 = singles.tile([P, 8, 512], BF16)
        nc.sync.dma_start(out=xt0, in_=xT[:, 0])

        def load_w(param):
            t = singles.tile([P, 8, DL], BF16, tag=param.name + "_sb")
            nc.sync.dma_start(out=t, in_=param[:, :, :])
            return t

        wq_sb = load_w(wq)
        wk_sb = load_w(wk)
        hmask_sb = singles.tile([P, HL], BF16)
        nc.sync.dma_start(out=hmask_sb, in_=hmask[:, :])
        wv_sb = load_w(wv)
        wck_sb = load_w(wck)
        wcv_sb = load_w(wcv)
        hsel_sb = singles.tile([HL, P], BF16)
        nc.sync.dma_start(out=hsel_sb, in_=hsel[:, :])
        ident_sb = singles.tile([P, P], BF16)
        nc.sync.dma_start(out=ident_sb, in_=ident[:, :])
        rotm_sb = singles.tile([P, P], BF16)
        nc.sync.dma_start(out=rotm_sb, in_=rotm[:, :])

        eps2 = singles.tile([HL, 1], F32)
        nc.vector.memset(eps2, EPS)

        # Normed/roped activations in transposed layout.
        qTn = singles.tile([P, N], BF16)
        kTn = singles.tile([P, NK], BF16)
        # V natural layout, per kc per head a 128-wide stationary block:
        # h0: [v(64) | ones(64)], h1: [ones(64) | v(64)]. The ones columns
        # land the softmax denominator on the PSUM rows opposite the o rows.
        v_all = singles.tile([P, KC, 2 * P], BF16)
        nc.vector.memset(v_all, 1.0)
        # Rope tables + rsqrt broadcasts live across both phases.
        cq_sb = singles.tile([P, N], BF16)
        sq_sb = singles.tile([P, N], BF16)
        ckc_sb = singles.tile([P, NK], BF16)
        cks_sb = singles.tile([P, NK], BF16)
        rsbq = singles.tile([P, NCH, 512], BF16)   # q rsqrt, per chunk
        rsbk = singles.tile([P, N], BF16)          # k-self rsqrt, wide
        rsbck = singles.tile([P, N], BF16)         # k-cross rsqrt, wide

        def rope_ops(eng, pool, dst, col0, w, tabC, tabS, tab0, rsb_ap,
                     rot_ps=None):
            """Yield the 4 DVE ops of one in-place rope over
            dst[:, col0:col0+w]. Caller drives the generator to schedule.
            rotate-half: SBUF->SBUF DMA by default; if rot_ps (a PSUM pool)
            is given, a PE matmul with the rot permutation instead -- DMAs
            are BLOCKED while an AllToAll is in flight, so fillers that run
            concurrently with collectives must not depend on DMAs."""
            sl = slice(col0, col0 + w)
            tsl = slice(tab0, tab0 + w)
            m1 = pool.tile([P, w], BF16, tag=f"m1_{w}", name="m1")
            if rot_ps is None:
                t1r = pool.tile([P, w], BF16, tag=f"t1r_{w}", name="t1r")
                eng.tensor_mul(m1, dst[:, sl], tabC[:, tsl])
                for h in range(HL):
                    b = h * DH
                    nc.sync.dma_start(out=t1r[b:b + 32, :],
                                      in_=dst[b + 32:b + 64, sl])
                    nc.sync.dma_start(out=t1r[b + 32:b + 64, :],
                                      in_=dst[b:b + 32, sl])
            else:
                t1r = rot_ps.tile([P, w], F32, tag="zp", name="t1r_ps")
                nc.tensor.matmul(t1r, rotm_sb, dst[:, sl],
                                 start=True, stop=True)
                eng.tensor_mul(m1, dst[:, sl], tabC[:, tsl])
            yield
            r1 = pool.tile([P, w], BF16, tag=f"r1_{w}", name="r1")
            eng.tensor_mul(r1, t1r, tabS[:, tsl])
            yield
            s2 = pool.tile([P, w], BF16, tag=f"s2_{w}", name="s2")
            eng.tensor_add(s2, m1, r1)
            yield
            eng.tensor_mul(dst[:, sl], s2, rsb_ap)

        def run_rope(gen):
            for _ in gen:
                pass

        # ---------------- phase 1: projections + RMSNorm + RoPE ------------
        with tc.tile_pool(name="xy", bufs=3) as xy, \
                tc.tile_pool(name="p1big", bufs=1) as p1big, \
                tc.tile_pool(name="p1work", bufs=4) as p1work, \
                tc.tile_pool(name="krope", bufs=2) as krope, \
                tc.tile_pool(name="proj_ps", bufs=2, space="PSUM") as proj_ps, \
                tc.tile_pool(name="ssq_ps", bufs=2, space="PSUM") as ssq_ps, \
                tc.tile_pool(name="trps", bufs=2, space="PSUM") as trps, \
                tc.tile_pool(name="rsb_ps", bufs=2, space="PSUM") as rsb_ps:

            vT_sb = p1big.tile([P, NK], BF16)
            ssq_all = p1big.tile([HL, 3 * N], F32)
            rs_all = p1big.tile([HL, 3 * N], BF16)

            def rsb_emit(pi, t, dst_ap):
                rp = rsb_ps.tile([P, 512], F32, tag="rsb")
                o = pi * N + t * 512
                nc.tensor.matmul(rp, hsel_sb, rs_all[:, o:o + 512],
                                 start=True, stop=True)
                nc.scalar.activation(out=dst_ap, in_=rp, func=AF.Copy)

            def finish_rs_qk():
                # one Ln + one Exp over q+k mean-squares (one table set pair)
                sl = slice(0, 2 * N)
                nc.scalar.activation(out=ssq_all[:, sl], in_=ssq_all[:, sl],
                                     func=AF.Ln, bias=eps2)
                nc.scalar.activation(out=rs_all[:, sl], in_=ssq_all[:, sl],
                                     func=AF.Exp, scale=-0.5)
                for t in range(4):
                    rsb_emit(0, t, rsbq[:, t, :])
                for t in range(4):
                    rsb_emit(1, t, rsbk[:, t * 512:(t + 1) * 512])

            def finish_rs_ck():
                sl = slice(2 * N, 3 * N)
                nc.scalar.activation(out=ssq_all[:, sl], in_=ssq_all[:, sl],
                                     func=AF.Ln, bias=eps2)
                nc.scalar.activation(out=rs_all[:, sl], in_=ssq_all[:, sl],
                                     func=AF.Exp, scale=-0.5)
                for t in range(4):
                    rsb_emit(2, t, rsbck[:, t * 512:(t + 1) * 512])

            ssq_pend = []
            ssq_done = [0, 0, 0]

            def drain_ssq(keep):
                # mean-square matmuls trail their projection by ~2 slots so
                # the ACT/DVE chains have drained (no in-order PE stall).
                while len(ssq_pend) > keep:
                    qsq, off, pi = ssq_pend.pop(0)
                    sp = ssq_ps.tile([HL, 512], F32, tag="ssq")
                    nc.tensor.matmul(sp, hmask_sb, qsq, start=True, stop=True)
                    nc.scalar.activation(out=ssq_all[:, off:off + 512], in_=sp,
                                         func=AF.Copy)
                    ssq_done[pi] += 1
                    if ssq_done[pi] == 4 and pi == 1:
                        finish_rs_qk()
                    elif ssq_done[pi] == 4 and pi == 2:
                        finish_rs_ck()

            def proj(w_sb, src, dst_ap, sq_info):
                ps = proj_ps.tile([P, 512], F32, tag="proj")
                for f in range(8):
                    nc.tensor.matmul(ps, w_sb[:, f, :], src[:, f, :],
                                     start=(f == 0), stop=(f == 7))
                if sq_info is None:
                    nc.vector.tensor_copy(dst_ap, ps)  # V path
                else:
                    nc.scalar.activation(out=dst_ap, in_=ps, func=AF.Copy)
                    qsq = p1work.tile([P, 512], BF16, tag="qsq")
                    nc.vector.tensor_mul(qsq, dst_ap, dst_ap)
                    ssq_pend.append((qsq, sq_info[0], sq_info[1]))

            def transpose_group(g):
                trp = trps.tile([P, 4, P], BF16, tag="trp")
                for i in range(4):
                    nc.tensor.transpose(trp[:, i, :],
                                        vT_sb[:, (4 * g + i) * P:(4 * g + i + 1) * P],
                                        ident_sb)
                sl4 = slice(4 * g, 4 * g + 4)
                nc.vector.tensor_copy(v_all[:, sl4, 0:DH], trp[:, :, 0:DH])
                nc.vector.tensor_copy(v_all[:, sl4, 3 * DH:4 * DH],
                                      trp[:, :, DH:2 * DH])

            for t in range(4):
                if t == 0:
                    xt = xt0
                else:
                    xt = xy.tile([P, 8, 512], BF16, tag="xy")
                    nc.sync.dma_start(out=xt, in_=xT[:, t])
                cs = slice(t * 512, (t + 1) * 512)
                proj(wq_sb, xt, qTn[:, cs], (t * 512, 0))
                proj(wk_sb, xt, kTn[:, cs], (N + t * 512, 1))
                proj(wv_sb, xt, vT_sb[:, cs], None)
                drain_ssq(2)

            # rope tables: loaded after x (startup DMA bandwidth goes to x).
            nc.sync.dma_start(out=cq_sb, in_=cq[:, :])
            nc.sync.dma_start(out=sq_sb, in_=sq[:, :])
            nc.sync.dma_start(out=ckc_sb, in_=ckc[:, :])
            nc.sync.dma_start(out=cks_sb, in_=cks[:, :])

            for t in range(4):
                yt = xy.tile([P, 8, 512], BF16, tag="xy")
                nc.sync.dma_start(out=yt, in_=yT[:, t])
                cs = slice(N + t * 512, N + (t + 1) * 512)
                proj(wck_sb, yt, kTn[:, cs], (2 * N + t * 512, 2))
                proj(wcv_sb, yt, vT_sb[:, cs], None)
                drain_ssq(2)
                if t == 2:
                    # q0 + wide k-self rope on the DVE, under y-proj PE work
                    run_rope(rope_ops(nc.vector, qrope, qTn, 0, 512,
                                      cq_sb, sq_sb, 0, rsbq[:, 0, :]))
                    run_rope(rope_ops(nc.vector, krope, kTn, 0, N,
                                      ckc_sb, cks_sb, 0, rsbk[:, :]))
                transpose_group(t)
            drain_ssq(0)
            for g in range(4, 8):
                transpose_group(g)
            # wide k-cross rope: attention only needs it from kc16 (~20us in)
            run_rope(rope_ops(nc.vector, krope, kTn, N, N,
                              ckc_sb, cks_sb, N, rsbck[:, :]))

        # ---------------- phase 2: attention + pipelined A2A + out-proj ----
        with tc.tile_pool(name="pv_ps", bufs=4, space="PSUM") as pv_ps, \
                tc.tile_pool(name="st_ps", bufs=2, space="PSUM") as st_ps, \
                tc.tile_pool(name="zp_ps", bufs=2, space="PSUM") as zp_ps, \
                tc.tile_pool(name="p2work", bufs=6) as p2work, \
                tc.tile_pool(name="p2out", bufs=2) as p2out:

            wo_sb = p2out.tile([P, 8, D], BF16, tag="wo_sb", bufs=1)
            nc.sync.dma_start(out=wo_sb, in_=wo[:, :, :])
            bo_sb = p2out.tile([1, D], BF16, tag="bo_sb", bufs=1)
            nc.sync.dma_start(out=bo_sb, in_=bo[0:1, :])
            bo_b = p2out.tile([P, D], BF16, tag="bo_b", bufs=1)
            nc.gpsimd.partition_broadcast(bo_b[0:P, :], bo_sb[0:1, :],
                                          channels=P)

            for c in range(NCH):
                cs = slice(c * CQ, (c + 1) * CQ)
                pv = [pv_ps.tile([P, CQ], F32, tag="pv", name=f"pv{c}_{h}")
                      for h in range(HL)]
                # q rope for chunk c+1, interleaved one op per kc below.
                # PE-based rotate-half: no DMA (DMAs stall during A2As).
                if c < NCH - 1:
                    filler = rope_ops(nc.vector, qrope, qTn, (c + 1) * CQ,
                                      512, cq_sb, sq_sb, (c + 1) * CQ,
                                      rsbq[:, c + 1, :], rot_ps=zp_ps)
                else:
                    filler = None

                def pv_mm(kc, h, e):
                    nc.tensor.matmul(
                        pv[h], v_all[:, kc, h * P:(h + 1) * P], e,
                        start=(kc == 0), stop=(kc == KC - 1))

                es_prev = None
                for kc in range(KC):
                    sts = [st_ps.tile([P, CQ], F32, tag="st",
                                      name=f"st{c}_{kc}_{h}")
                           for h in range(HL)]
                    for h in range(HL):
                        hs = slice(h * DH, (h + 1) * DH)
                        nc.tensor.matmul(sts[h], kTn[hs, kc * P:(kc + 1) * P],
                                         qTn[hs, cs], start=True, stop=True)
                    # h1's PV deferred one kc (fills the PE queue); h0's PV
                    # waits on THIS kc's ACT exp -- that dependency paces the
                    # PE at ~1.3us/kc when the DVFS grant is high, which keeps
                    # it off the sustained-full-clock profile that trips the
                    # activity throttle into a long 50% clamp (measured: a
                    # fully dense 1.05us/kc stream gets clamped within ~8us).
                    if es_prev is not None:
                        pv_mm(kc - 1, 1, es_prev)
                    es = []
                    for h in range(HL):
                        e = p2work.tile([P, CQ], BF16, tag="es", bufs=6)
                        if h == 0:
                            nc.scalar.activation(out=e, in_=sts[h],
                                                 func=AF.Exp, scale=SCALE)
                        else:
                            # Schraudolph bf16 exp on the DVE
                            nc.vector.tensor_scalar(
                                out=e.bitcast(I16), in0=sts[h],
                                scalar1=EXA, scalar2=EXB,
                                op0=mybir.AluOpType.mult,
                                op1=mybir.AluOpType.add)
                        es.append(e)
                    pv_mm(kc, 0, es[0])
                    es_prev = es[1]
                    if filler is not None and kc % 2 == 1:
                        if next(filler, StopIteration) is StopIteration:
                            filler = None
                pv_mm(KC - 1, 1, es_prev)

                # normalize. denominators sit on the PSUM rows opposite the
                # o rows (64 copies: h0 on pv0[64:128], h1 on pv1[0:64]).
                # Reciprocal: magic-seed Newton (2 iters) on the DVE -- no
                # ACT table switches, no DMAs (which stall during A2As).
                # The recip crosses to the o-rows via two tiny PE matmuls
                # with identity blocks (bf16 moving, 512 cols each).
                oT = p2work.tile([P, CQ], BF16, tag="oT", bufs=2)
                d2 = p2work.tile([P, CQ], F32, tag="d2", bufs=2)
                xw = p2work.tile([P, CQ], F32, tag="xw", bufs=2)
                tw = p2work.tile([P, CQ], F32, tag="tw", bufs=2)
                uw = p2work.tile([P, CQ], F32, tag="uw", bufs=2)
                xr = p2work.tile([P, CQ], BF16, tag="xr", bufs=2)
                nc.vector.tensor_copy(d2[0:DH, :], pv[1][0:DH, :])
                nc.vector.tensor_copy(d2[DH:P, :], pv[0][DH:P, :])
                nc.vector.tensor_scalar(
                    out=xw.bitcast(I32), in0=d2.bitcast(I32),
                    scalar1=-1, scalar2=RMAGIC,
                    op0=mybir.AluOpType.mult, op1=mybir.AluOpType.add)
                for it in range(2):
                    nc.vector.tensor_mul(tw, d2, xw)
                    nc.vector.tensor_scalar(
                        out=uw, in0=tw, scalar1=-1.0, scalar2=2.0,
                        op0=mybir.AluOpType.mult, op1=mybir.AluOpType.add)
                    nc.vector.tensor_mul(xr if it == 1 else xw,
                                         xw, uw)
                rdb = zp_ps.tile([P, CQ], F32, tag="zp", name=f"rdb{c}")
                nc.tensor.matmul(rdb[0:DH, :], ident_sb[DH:P, DH:P],
                                 xr[DH:P, :], start=True, stop=True)
                nc.tensor.matmul(rdb[DH:P, :], ident_sb[0:DH, 0:DH],
                                 xr[0:DH, :], start=True, stop=True)
                # DVE can read only ONE operand from PSUM: evict rdb first
                rdbs = p2work.tile([P, CQ], BF16, tag="rdbs", bufs=2)
                nc.vector.tensor_copy(rdbs, rdb)
                nc.vector.tensor_mul(oT[0:DH, :], pv[0][0:DH, :],
                                     rdbs[0:DH, :])
                nc.vector.tensor_mul(oT[DH:P, :], pv[1][DH:P, :],
                                     rdbs[DH:P, :])

                # A2A for this chunk: dest core j gets positions j*64..+64.
                nc.sync.dma_start(
                    out=a2a_in[c][:, :, :].transpose([1, 0, 2]), in_=oT[:, :])
                if c == NCH - 1:
                    # out-proj input loads for everything already landed,
                    # issued BEFORE the last trigger: DMAs stall while an
                    # A2A is in flight, and pair 0's matmuls must run
                    # INSIDE the final A2A's flight window.
                    of0 = p2out.tile([P, NCORES, P], BF16, tag="of",
                                     name="of0")
                    of1 = p2out.tile([P, NCORES, P], BF16, tag="of",
                                     name="of1")
                    for half, cc_ in ((0, 0), (1, 1)):
                        nc.sync.dma_start(
                            out=of0[:, :, half * CPC:(half + 1) * CPC],
                            in_=a2a_out[cc_][:, :, :].transpose([1, 0, 2]))
                    nc.sync.dma_start(
                        out=of1[:, :, 0:CPC],
                        in_=a2a_out[2][:, :, :].transpose([1, 0, 2]))
                nc.gpsimd.collective_compute(
                    "AllToAll", mybir.AluOpType.bypass,
                    replica_groups=[list(range(NCORES))],
                    ins=[a2a_in[c][:, :, :]],
                    outs=[a2a_out[c][:, :, :]],
                )

            # out-projection tail: only chunk 3's slice still needs loading
            # (after its A2A); pair 0's matmuls overlap that A2A's flight.
            nc.sync.dma_start(
                out=of1[:, :, CPC:2 * CPC],
                in_=a2a_out[3][:, :, :].transpose([1, 0, 2]))

            def outproj(pair, of):
                for nn in range(2):
                    zp = zp_ps.tile([P, 512], F32, tag="zp")
                    for j in range(NCORES):
                        nc.tensor.matmul(zp, of[:, j, :],
                                         wo_sb[:, j, nn * 512:(nn + 1) * 512],
                                         start=(j == 0), stop=(j == NCORES - 1))
                    zs = p2out.tile([P, 512], F32, tag="zs")
                    nc.vector.tensor_add(zs, zp,
                                         bo_b[:, nn * 512:(nn + 1) * 512])
                    nc.sync.dma_start(out=out_ext[pair * P:(pair + 1) * P,
                                                  nn * 512:(nn + 1) * 512],
                                      in_=zs)

            outproj(0, of0)
            outproj(1, of1)
    return nc


def _bf16(a):
    return np.ascontiguousarray(a).astype(ml_dtypes.bfloat16)


def _rope_tables(npos, pos0, g_first, g_second, n_first):
    """Tables [128, npos] for transposed-layout rope with g folded in.

    Row j (within a head, duplicated for 2 local heads):
      out[j] = t[j]*C[j] + t[sigma(j)]*S[j]
      j <  32: C[j]=g[j]*cos[n,j],     S[j]=-g[j+32]*sin[n,j]
      j >= 32: C[j]=g[j]*cos[n,j-32],  S[j]=+g[j-32]*sin[n,j-32]
    g switches from g_first to g_second at position n_first.
    """
    inv = 1.0 / (ROPE_BASE ** (np.arange(0, DH, 2, dtype=np.float64) / DH))
    pos = np.arange(pos0, pos0 + npos, dtype=np.float64)
    ang = pos[:, None] * inv[None, :]          # [npos, 32]
    cos = np.cos(ang).T                         # [32, npos]
    sin = np.sin(ang).T
    C = np.zeros((DH, npos), np.float64)
    S = np.zeros((DH, npos), np.float64)
    g = np.zeros((DH, npos), np.float64)
    g[:, :n_first] = np.asarray(g_first, np.float64)[:, None]
    if n_first < npos:
        g[:, n_first:] = np.asarray(g_second, np.float64)[:, None]
    C[:32] = cos
    C[32:] = cos
    C *= g
    S[:32] = -sin
    S[32:] = sin
    Srot = np.concatenate([g[32:], g[:32]], axis=0)  # g[sigma(j)]
    S *= Srot
    C2 = np.concatenate([C, C], axis=0)  # duplicate for 2 local heads
    S2 = np.concatenate([S, S], axis=0)
    return _bf16(C2), _bf16(S2)


_NC_CACHE = None


def kernel(x, y, W_qkv, W_ckv, W_out, b_out, g_q, g_k, g_ck, n_heads):
    global LAST_RESULT, _NC_CACHE
    x = np.asarray(x, np.float32)
    y = np.asarray(y, np.float32)
    W_qkv = np.asarray(W_qkv, np.float32)
    W_ckv = np.asarray(W_ckv, np.float32)
    W_out = np.asarray(W_out, np.float32)
    b_out = np.asarray(b_out, np.float32)

    def _prearr_x(a):
        # a [2048 pos, 1024 feat] -> [p, chunk, f, 512]:
        # element (f*128+p, c*512+ns) lands at [p, c, f, ns]
        return _bf16(a.T.reshape(8, P, 4, 512).transpose(1, 2, 0, 3))

    xT = _prearr_x(x[0])
    yT = _prearr_x(y[0])
    Wq, Wk, Wv = (W_qkv[:, i * D:(i + 1) * D] for i in range(3))
    Wck, Wcv = (W_ckv[:, i * D:(i + 1) * D] for i in range(2))

    def _prearr(w):
        # [1024, C] row f*128+p -> [p, f, c]: contiguous per-partition DMAs
        return _bf16(w.reshape(8, P, -1).transpose(1, 0, 2))

    woh = _prearr(W_out)
    boh = _bf16(b_out[None, :])

    cqh, sqh = _rope_tables(N, 0, g_q, g_q, N)
    ckch, cksh = _rope_tables(NK, 0, g_k, g_ck, N)
    hm = np.zeros((P, HL), np.float32)
    for h in range(HL):
        hm[h * DH:(h + 1) * DH, h] = 1.0 / DH
    hmh = _bf16(hm)
    hs = np.zeros((HL, P), np.float32)
    for h in range(HL):
        hs[h, h * DH:(h + 1) * DH] = 1.0
    hsh = _bf16(hs)
    idh = _bf16(np.eye(P, dtype=np.float32))
    # rotate-half permutation as a matmul: rot(t)[m] = t[sigma(m)],
    # sigma = swap 32-halves within each 64-dim head; rotm[k, m] = (k==sigma(m))
    rm = np.zeros((P, P), np.float32)
    for h in range(HL):
        b = h * DH
        for m in range(32):
            rm[b + 32 + m, b + m] = 1.0
            rm[b + m, b + 32 + m] = 1.0
    rmh = _bf16(rm)

    in_maps = []
    for c in range(NCORES):
        sl = slice(c * DL, (c + 1) * DL)
        in_maps.append({
            "xT": xT, "yT": yT,
            "wq": _prearr(Wq[:, sl]), "wk": _prearr(Wk[:, sl]),
            "wv": _prearr(Wv[:, sl]), "wck": _prearr(Wck[:, sl]),
            "wcv": _prearr(Wcv[:, sl]),
            "wo": woh, "bo": boh,
            "cq": cqh, "sq": sqh, "ckc": ckch, "cks": cksh,
            "hmask": hmh, "hsel": hsh, "ident": idh, "rotm": rmh,
        })

    if _NC_CACHE is None:
        _NC_CACHE = build_nc()
        if not _NC_CACHE.is_finalized():
            _NC_CACHE.finalize()
    nc = _NC_CACHE

    res = run_bass_kernel_spmd(
        nc, in_maps, core_ids=list(range(NCORES)),
        trace=bool(os.environ.get("BASS_TRACE")),
    )
    LAST_RESULT = res
    # out_ext rows on core j: pair*128 + half*64 + cc
    #   <-> global position (pair*2 + half)*512 + j*64 + cc
    out = np.empty((N, D), np.float32)
    for j in range(NCORES):
        o = np.asarray(res.results[j]["out"], np.float32)
        for ch in range(NCH):
            out[ch * CQ + j * CPC:ch * CQ + (j + 1) * CPC] = \
                o[ch * CPC:(ch + 1) * CPC]
    return out[None, :, :]


# revision 18
# speedup vs baseline: 1.5232x; 1.5232x over previous
"""Distributed Trainium2 kernel for nn_Attn_77970836292156.

Cross-attention block: fused QKV projection + per-head RMSNorm + RoPE +
bf16 SDPA (4096 keys = 2048 self + 2048 cross) + output projection.

Sharding: tensor-parallel on heads. 16 heads / 8 cores = 2 heads per core.
W_qkv / W_ckv column-sharded by head; every core holds full x, y (transposed,
bf16). Attention runs fully local per core in a transposed layout
(head-dims on partitions, positions on the free axis). An AllToAll converts
head-sharding -> sequence-sharding, then each core applies the full W_out to
its position slice (row-sharded matmul accumulated over all 1024 dims).

Structure (v3 -- rebuilt from trace analysis):
- Attention runs in FOUR 512-query chunks, each with its own (128KB)
  AllToAll, so collectives pipeline under later chunks' compute and only
  the last chunk's A2A is exposed (measured ~5-7us once cores are in
  lockstep, vs ~25us for the old single 256KB A2A at the end).
- PSUM: pv accumulators [128,512] x2 heads double-buffered (4 banks) +
  QK score tiles (2 banks) + out-proj accumulators (2 banks) = 8 banks,
  so consecutive chunks never stall on each other's normalize.
- Every matmul has a 128-column stationary operand => fast-weight-load
  stays enabled (a 65-wide PV stationary disables FWL and costs
  ~100ns/matmul). The PV stationary per head is [v(64) | ones(64)] (or
  mirrored), so the PSUM rows opposite the o-rows accumulate 64 copies
  of the softmax denominator: the reciprocal (ACT Ln + Exp(-x), ordered
  Ln/Ln then Exp/Exp = 2 table loads) runs on 64 partitions directly and
  needs only a partition-shift DMA, no broadcast primitive.
- Softmax exp split across engines per head: h0 on ACT (table exp), h1 on
  DVE via a Schraudolph bit-trick (bits = trunc(score*a + b) as int16,
  reinterpreted bf16).
- RMSNorm rsqrt batched q+k (one Ln + one Exp over [2,4096] as soon as
  the x-side mean-squares land, during the y projections) + ck at
  phase-1 end: 4 ACT table loads total in phase 1.
- RoPE all on DVE (GpSimd tensor ops measured 1.4us/op -- useless).
  K is roped in two FULL-WIDTH [128,2048] passes (per-element DVE cost
  ~5x lower than 512-wide slices); q is roped per 512-chunk: q0 in the
  phase-1 foreground, q1/q2/q3 interleaved one-op-per-kc into attention
  chunks 0/1/2 so they never delay the chunk that needs them.
- A2A staging is ONE transposed-AP DMA per chunk; out-proj loads are one
  DMA per chunk emitted AFTER the last collective so the in-order SP
  queue can never wedge on a collective wait. Both out-projections are
  emitted after chunk 3: pair 0's matmuls execute inside the final A2A's
  flight window, so the exposed tail is ~(A2A + pair-1 matmuls) only.
- x/y are consumed through a 3-deep chunk pool with the x0 load issued
  FIRST (before the small weight loads): the first projection starts
  ~7us in instead of ~20us.
"""

import os

import numpy as np
import ml_dtypes

import concourse.bass as bass
import concourse.tile as tile
from concourse import bacc, mybir
from concourse.bass_utils import run_bass_kernel_spmd

BF16 = mybir.dt.bfloat16
F32 = mybir.dt.float32
I16 = mybir.dt.int16
I32 = mybir.dt.int32
AF = mybir.ActivationFunctionType
RMAGIC = 0x7EF127EA  # f32 reciprocal Newton seed: x0_bits = RMAGIC - d_bits

# Problem constants (hardcoded per spec).
N = 2048        # query positions
M = 2048        # cross positions
NK = N + M      # total keys
D = 1024        # model dim
H = 16          # heads
DH = 64         # head dim
HL = 2          # heads per core
DL = HL * DH    # local head dims = 128
P = 128
NCORES = 8
EPS = 1e-6
ROPE_BASE = 10000.0
SCALE = 0.125   # 1/sqrt(64)
KC = NK // P    # 32 key chunks of 128
NCH = 4         # query chunks
CQ = N // NCH   # 512 queries per chunk
CPC = CQ // NCORES  # 64 positions per core per chunk

# Schraudolph exp constants for bf16 bits = trunc(score*EXA + EXB):
#   bits = 128*(score*SCALE*log2 e) + 127*128 - 5.5 (minimax centering)
#   + 0.5 (truncation compensation)
EXA = SCALE * 128.0 * 1.4426950408889634
EXB = 16251.0

LAST_RESULT = None  # test harness reads exec_time_ns from here


def build_nc():
    nc = bacc.Bacc()

    # ---------------- DRAM parameters ----------------
    # x/y arrive host-prearranged chunk-major [p, chunk, f, 512] so each
    # position-chunk load is one contiguous 8KB run per partition.
    xT = nc.declare_dram_parameter("xT", [P, 4, 8, 512], BF16, isOutput=False)
    yT = nc.declare_dram_parameter("yT", [P, 4, 8, 512], BF16, isOutput=False)
    wq = nc.declare_dram_parameter("wq", [P, 8, DL], BF16, isOutput=False)
    wk = nc.declare_dram_parameter("wk", [P, 8, DL], BF16, isOutput=False)
    wv = nc.declare_dram_parameter("wv", [P, 8, DL], BF16, isOutput=False)
    wck = nc.declare_dram_parameter("wck", [P, 8, DL], BF16, isOutput=False)
    wcv = nc.declare_dram_parameter("wcv", [P, 8, DL], BF16, isOutput=False)
    wo = nc.declare_dram_parameter("wo", [P, 8, D], BF16, isOutput=False)
    bo = nc.declare_dram_parameter("bo", [1, D], BF16, isOutput=False)
    cq = nc.declare_dram_parameter("cq", [P, N], BF16, isOutput=False)
    sq = nc.declare_dram_parameter("sq", [P, N], BF16, isOutput=False)
    ckc = nc.declare_dram_parameter("ckc", [P, NK], BF16, isOutput=False)
    cks = nc.declare_dram_parameter("cks", [P, NK], BF16, isOutput=False)
    hmask = nc.declare_dram_parameter("hmask", [P, HL], BF16, isOutput=False)
    hsel = nc.declare_dram_parameter("hsel", [HL, P], BF16, isOutput=False)
    ident = nc.declare_dram_parameter("ident", [P, P], BF16, isOutput=False)
    rotm = nc.declare_dram_parameter("rotm", [P, P], BF16, isOutput=False)
    out_ext = nc.declare_dram_parameter("out", [2 * P, D], F32, isOutput=True)

    # A2A bounce buffers, one pair per q chunk (collectives can't touch I/O
    # tensors; separate tensors keep chunk deps independent).
    a2a_in = [nc.dram_tensor(f"a2a_in{c}", [NCORES, P, CPC], BF16)
              for c in range(NCH)]
    a2a_out = [nc.dram_tensor(f"a2a_out{c}", [NCORES, P, CPC], BF16)
               for c in range(NCH)]

    with tile.TileContext(nc) as tc, \
            tc.tile_pool(name="singles", bufs=1) as singles, \
            tc.tile_pool(name="qrope", bufs=2) as qrope:

        # ---------------- static SBUF loads (x chunk 0 issued first) -------
        xt0 = singles.tile([P, 8, 512], BF16)
        nc.sync.dma_start(out=xt0, in_=xT[:, 0])

        def load_w(param):
            t = singles.tile([P, 8, DL], BF16, tag=param.name + "_sb")
            nc.sync.dma_start(out=t, in_=param[:, :, :])
            return t

        wq_sb = load_w(wq)
        wk_sb = load_w(wk)
        hmask_sb = singles.tile([P, HL], BF16)
        nc.sync.dma_start(out=hmask_sb, in_=hmask[:, :])
        wv_sb = load_w(wv)
        wck_sb = load_w(wck)
        wcv_sb = load_w(wcv)
        hsel_sb = singles.tile([HL, P], BF16)
        nc.sync.dma_start(out=hsel_sb, in_=hsel[:, :])
        ident_sb = singles.tile([P, P], BF16)
        nc.sync.dma_start(out=ident_sb, in_=ident[:, :])
        rotm_sb = singles.tile([P, P], BF16)
        nc.sync.dma_start(out=rotm_sb, in_=rotm[:, :])

        eps2 = singles.tile([HL, 1], F32)
        nc.vector.memset(eps2, EPS)

        # Normed/roped activations in transposed layout.
        qTn = singles.tile([P, N], BF16)
        kTn = singles.tile([P, NK], BF16)
        # V natural layout, per kc per head a 128-wide stationary block:
        # h0: [v(64) | ones(64)], h1: [ones(64) | v(64)]. The ones columns
        # land the softmax denominator on the PSUM rows opposite the o rows.
        v_all = singles.tile([P, KC, 2 * P], BF16)
        nc.vector.memset(v_all, 1.0)
        # Rope tables + rsqrt broadcasts live across both phases.
        cq_sb = singles.tile([P, N], BF16)
        sq_sb = singles.tile([P, N], BF16)
        ckc_sb = singles.tile([P, NK], BF16)
        cks_sb = singles.tile([P, NK], BF16)
        rsbq = singles.tile([P, NCH, 512], BF16)   # q rsqrt, per chunk
        rsbk = singles.tile([P, N], BF16)          # k-self rsqrt, wide
        rsbck = singles.tile([P, N], BF16)         # k-cross rsqrt, wide

        def rope_ops(eng, pool, dst, col0, w, tabC, tabS, tab0, rsb_ap,
                     rot_ps=None):
            """Yield the 4 DVE ops of one in-place rope over
            dst[:, col0:col0+w]. Caller drives the generator to schedule.
            rotate-half: SBUF->SBUF DMA by default; if rot_ps (a PSUM pool)
            is given, a PE matmul with the rot permutation instead -- DMAs
            are BLOCKED while an AllToAll is in flight, so fillers that run
            concurrently with collectives must not depend on DMAs."""
            sl = slice(col0, col0 + w)
            tsl = slice(tab0, tab0 + w)
            m1 = pool.tile([P, w], BF16, tag=f"m1_{w}", name="m1")
            if rot_ps is None:
                t1r = pool.tile([P, w], BF16, tag=f"t1r_{w}", name="t1r")
                eng.tensor_mul(m1, dst[:, sl], tabC[:, tsl])
                for h in range(HL):
                    b = h * DH
                    nc.sync.dma_start(out=t1r[b:b + 32, :],
                                      in_=dst[b + 32:b + 64, sl])
                    nc.sync.dma_start(out=t1r[b + 32:b + 64, :],
                                      in_=dst[b:b + 32, sl])
            else:
                t1r = rot_ps.tile([P, w], F32, tag="zp", name="t1r_ps")
                nc.tensor.matmul(t1r, rotm_sb, dst[:, sl],
                                 start=True, stop=True)
                eng.tensor_mul(m1, dst[:, sl], tabC[:, tsl])
            yield
            r1 = pool.tile([P, w], BF16, tag=f"r1_{w}", name="r1")
            eng.tensor_mul(r1, t1r, tabS[:, tsl])
            yield
            s2 = pool.tile([P, w], BF16, tag=f"s2_{w}", name="s2")
            eng.tensor_add(s2, m1, r1)
            yield
            eng.tensor_mul(dst[:, sl], s2, rsb_ap)

        def run_rope(gen):
            for _ in gen:
                pass

        # ---------------- phase 1: projections + RMSNorm + RoPE ------------
        with tc.tile_pool(name="xy", bufs=3) as xy, \
                tc.tile_pool(name="p1big", bufs=1) as p1big, \
                tc.tile_pool(name="p1work", bufs=4) as p1work, \
                tc.tile_pool(name="krope", bufs=2) as krope, \
                tc.tile_pool(name="proj_ps", bufs=2, space="PSUM") as proj_ps, \
                tc.tile_pool(name="ssq_ps", bufs=2, space="PSUM") as ssq_ps, \
                tc.tile_pool(name="trps", bufs=2, space="PSUM") as trps, \
                tc.tile_pool(name="rsb_ps", bufs=2, space="PSUM") as rsb_ps:

            vT_sb = p1big.tile([P, NK], BF16)
            ssq_all = p1big.tile([HL, 3 * N], F32)
            rs_all = p1big.tile([HL, 3 * N], BF16)

            def rsb_emit(pi, t, dst_ap):
                rp = rsb_ps.tile([P, 512], F32, tag="rsb")
                o = pi * N + t * 512
                nc.tensor.matmul(rp, hsel_sb, rs_all[:, o:o + 512],
                                 start=True, stop=True)
                nc.scalar.activation(out=dst_ap, in_=rp, func=AF.Copy)

            def finish_rs_qk():
                # one Ln + one Exp over q+k mean-squares (one table set pair)
                sl = slice(0, 2 * N)
                nc.scalar.activation(out=ssq_all[:, sl], in_=ssq_all[:, sl],
                                     func=AF.Ln, bias=eps2)
                nc.scalar.activation(out=rs_all[:, sl], in_=ssq_all[:, sl],
                                     func=AF.Exp, scale=-0.5)
                for t in range(4):
                    rsb_emit(0, t, rsbq[:, t, :])
                for t in range(4):
                    rsb_emit(1, t, rsbk[:, t * 512:(t + 1) * 512])

            def finish_rs_ck():
                sl = slice(2 * N, 3 * N)
                nc.scalar.activation(out=ssq_all[:, sl], in_=ssq_all[:, sl],
                                     func=AF.Ln, bias=eps2)
                nc.scalar.activation(out=rs_all[:, sl], in_=ssq_all[:, sl],
                                     func=AF.Exp, scale=-0.5)
                for t in range(4):
                    rsb_emit(2, t, rsbck[:, t * 512:(t + 1) * 512])

            ssq_pend = []
            ssq_done = [0, 0, 0]

            def drain_ssq(keep):
                # mean-square matmuls trail their projection by ~2 slots so
                # the ACT/DVE chains have drained (no in-order PE stall).
                while len(ssq_pend) > keep:
                    qsq, off, pi = ssq_pend.pop(0)
                    sp = ssq_ps.tile([HL, 512], F32, tag="ssq")
                    nc.tensor.matmul(sp, hmask_sb, qsq, start=True, stop=True)
                    nc.scalar.activation(out=ssq_all[:, off:off + 512], in_=sp,
                                         func=AF.Copy)
                    ssq_done[pi] += 1
                    if ssq_done[pi] == 4 and pi == 1:
                        finish_rs_qk()
                    elif ssq_done[pi] == 4 and pi == 2:
                        finish_rs_ck()

            def proj(w_sb, src, dst_ap, sq_info):
                ps = proj_ps.tile([P, 512], F32, tag="proj")
                for f in range(8):
                    nc.tensor.matmul(ps, w_sb[:, f, :], src[:, f, :],
                                     start=(f == 0), stop=(f == 7))
                if sq_info is None:
                    nc.vector.tensor_copy(dst_ap, ps)  # V path
                else:
                    nc.scalar.activation(out=dst_ap, in_=ps, func=AF.Copy)
                    qsq = p1work.tile([P, 512], BF16, tag="qsq")
                    nc.vector.tensor_mul(qsq, dst_ap, dst_ap)
                    ssq_pend.append((qsq, sq_info[0], sq_info[1]))

            def transpose_group(g):
                trp = trps.tile([P, 4, P], BF16, tag="trp")
                for i in range(4):
                    nc.tensor.transpose(trp[:, i, :],
                                        vT_sb[:, (4 * g + i) * P:(4 * g + i + 1) * P],
                                        ident_sb)
                sl4 = slice(4 * g, 4 * g + 4)
                nc.vector.tensor_copy(v_all[:, sl4, 0:DH], trp[:, :, 0:DH])
                nc.vector.tensor_copy(v_all[:, sl4, 3 * DH:4 * DH],
                                      trp[:, :, DH:2 * DH])

            for t in range(4):
                if t == 0:
                    xt = xt0
                else:
                    xt = xy.tile([P, 8, 512], BF16, tag="xy")
                    nc.sync.dma_start(out=xt, in_=xT[:, t])
                cs = slice(t * 512, (t + 1) * 512)
                proj(wq_sb, xt, qTn[:, cs], (t * 512, 0))
                proj(wk_sb, xt, kTn[:, cs], (N + t * 512, 1))
                proj(wv_sb, xt, vT_sb[:, cs], None)
                drain_ssq(2)

            # rope tables: loaded after x (startup DMA bandwidth goes to x).
            nc.sync.dma_start(out=cq_sb, in_=cq[:, :])
            nc.sync.dma_start(out=sq_sb, in_=sq[:, :])
            nc.sync.dma_start(out=ckc_sb, in_=ckc[:, :])
            nc.sync.dma_start(out=cks_sb, in_=cks[:, :])

            for t in range(4):
                yt = xy.tile([P, 8, 512], BF16, tag="xy")
                nc.sync.dma_start(out=yt, in_=yT[:, t])
                cs = slice(N + t * 512, N + (t + 1) * 512)
                proj(wck_sb, yt, kTn[:, cs], (2 * N + t * 512, 2))
                proj(wcv_sb, yt, vT_sb[:, cs], None)
                drain_ssq(2)
                if t == 2:
                    # q0 + wide k-self rope on the DVE, under y-proj PE work
                    run_rope(rope_ops(nc.vector, qrope, qTn, 0, 512,
                                      cq_sb, sq_sb, 0, rsbq[:, 0, :]))
                    run_rope(rope_ops(nc.vector, krope, kTn, 0, N,
                                      ckc_sb, cks_sb, 0, rsbk[:, :]))
                transpose_group(t)
            drain_ssq(0)
            for g in range(4, 8):
                transpose_group(g)
            # wide k-cross rope: attention only needs it from kc16 (~20us in)
            run_rope(rope_ops(nc.vector, krope, kTn, N, N,
                              ckc_sb, cks_sb, N, rsbck[:, :]))

        # ---------------- phase 2: attention + pipelined A2A + out-proj ----
        with tc.tile_pool(name="pv_ps", bufs=4, space="PSUM") as pv_ps, \
                tc.tile_pool(name="st_ps", bufs=2, space="PSUM") as st_ps, \
                tc.tile_pool(name="zp_ps", bufs=2, space="PSUM") as zp_ps, \
                tc.tile_pool(name="p2work", bufs=6) as p2work, \
                tc.tile_pool(name="p2out", bufs=2) as p2out:

            wo_sb = p2out.tile([P, 8, D], BF16, tag="wo_sb", bufs=1)
            nc.sync.dma_start(out=wo_sb, in_=wo[:, :, :])
            bo_sb = p2out.tile([1, D], BF16, tag="bo_sb", bufs=1)
            nc.sync.dma_start(out=bo_sb, in_=bo[0:1, :])
            bo_b = p2out.tile([P, D], BF16, tag="bo_b", bufs=1)
            nc.gpsimd.partition_broadcast(bo_b[0:P, :], bo_sb[0:1, :],
                                          channels=P)

            for c in range(NCH):
                cs = slice(c * CQ, (c + 1) * CQ)
                pv = [pv_ps.tile([P, CQ], F32, tag="pv", name=f"pv{c}_{h}")
                      for h in range(HL)]
                # q rope for chunk c+1, interleaved one op per kc below.
                # PE-based rotate-half: no DMA (DMAs stall during A2As).
                if c < NCH - 1:
                    filler = rope_ops(nc.vector, qrope, qTn, (c + 1) * CQ,
                                      512, cq_sb, sq_sb, (c + 1) * CQ,
                                      rsbq[:, c + 1, :], rot_ps=zp_ps)
                else:
                    filler = None

                def pv_mm(kc, h, e):
                    nc.tensor.matmul(
                        pv[h], v_all[:, kc, h * P:(h + 1) * P], e,
                        start=(kc == 0), stop=(kc == KC - 1))

                es_prev = None
                for kc in range(KC):
                    sts = [st_ps.tile([P, CQ], F32, tag="st",
                                      name=f"st{c}_{kc}_{h}")
                           for h in range(HL)]
                    for h in range(HL):
                        hs = slice(h * DH, (h + 1) * DH)
                        nc.tensor.matmul(sts[h], kTn[hs, kc * P:(kc + 1) * P],
                                         qTn[hs, cs], start=True, stop=True)
                    # previous kc's PVs fill the PE while this kc's exps run
                    if es_prev is not None:
                        pv_mm(kc - 1, 0, es_prev[0])
                        pv_mm(kc - 1, 1, es_prev[1])
                    es = []
                    for h in range(HL):
                        e = p2work.tile([P, CQ], BF16, tag="es", bufs=6)
                        if h == 0:
                            nc.scalar.activation(out=e, in_=sts[h],
                                                 func=AF.Exp, scale=SCALE)
                        else:
                            # Schraudolph bf16 exp on the DVE
                            nc.vector.tensor_scalar(
                                out=e.bitcast(I16), in0=sts[h],
                                scalar1=EXA, scalar2=EXB,
                                op0=mybir.AluOpType.mult,
                                op1=mybir.AluOpType.add)
                        es.append(e)
                    es_prev = es
                    if filler is not None and kc % 2 == 1:
                        if next(filler, StopIteration) is StopIteration:
                            filler = None
                pv_mm(KC - 1, 0, es_prev[0])
                pv_mm(KC - 1, 1, es_prev[1])

                # normalize. denominators sit on the PSUM rows opposite the
                # o rows (64 copies: h0 on pv0[64:128], h1 on pv1[0:64]).
                # Reciprocal: magic-seed Newton (2 iters) on the DVE -- no
                # ACT table switches, no DMAs (which stall during A2As).
                # The recip crosses to the o-rows via two tiny PE matmuls
                # with identity blocks (bf16 moving, 512 cols each).
                oT = p2work.tile([P, CQ], BF16, tag="oT", bufs=2)
                d2 = p2work.tile([P, CQ], F32, tag="d2", bufs=2)
                xw = p2work.tile([P, CQ], F32, tag="xw", bufs=2)
                tw = p2work.tile([P, CQ], F32, tag="tw", bufs=2)
                uw = p2work.tile([P, CQ], F32, tag="uw", bufs=2)
                xr = p2work.tile([P, CQ], BF16, tag="xr", bufs=2)
                nc.vector.tensor_copy(d2[0:DH, :], pv[1][0:DH, :])
                nc.vector.tensor_copy(d2[DH:P, :], pv[0][DH:P, :])
                nc.vector.tensor_scalar(
                    out=xw.bitcast(I32), in0=d2.bitcast(I32),
                    scalar1=-1, scalar2=RMAGIC,
                    op0=mybir.AluOpType.mult, op1=mybir.AluOpType.add)
                for it in range(2):
                    nc.vector.tensor_mul(tw, d2, xw)
                    nc.vector.tensor_scalar(
                        out=uw, in0=tw, scalar1=-1.0, scalar2=2.0,
                        op0=mybir.AluOpType.mult, op1=mybir.AluOpType.add)
                    nc.vector.tensor_mul(xr if it == 1 else xw,
                                         xw, uw)
                rdb = zp_ps.tile([P, CQ], F32, tag="zp", name=f"rdb{c}")
                nc.tensor.matmul(rdb[0:DH, :], ident_sb[DH:P, DH:P],
                                 xr[DH:P, :], start=True, stop=True)
                nc.tensor.matmul(rdb[DH:P, :], ident_sb[0:DH, 0:DH],
                                 xr[0:DH, :], start=True, stop=True)
                # DVE can read only ONE operand from PSUM: evict rdb first
                rdbs = p2work.tile([P, CQ], BF16, tag="rdbs", bufs=2)
                nc.vector.tensor_copy(rdbs, rdb)
                nc.vector.tensor_mul(oT[0:DH, :], pv[0][0:DH, :],
                                     rdbs[0:DH, :])
                nc.vector.tensor_mul(oT[DH:P, :], pv[1][DH:P, :],
                                     rdbs[DH:P, :])

                # A2A for this chunk: dest core j gets positions j*64..+64.
                nc.sync.dma_start(
                    out=a2a_in[c][:, :, :].transpose([1, 0, 2]), in_=oT[:, :])
                if c == NCH - 1:
                    # out-proj input loads for everything already landed,
                    # issued BEFORE the last trigger: DMAs stall while an
                    # A2A is in flight, and pair 0's matmuls must run
                    # INSIDE the final A2A's flight window.
                    of0 = p2out.tile([P, NCORES, P], BF16, tag="of",
                                     name="of0")
                    of1 = p2out.tile([P, NCORES, P], BF16, tag="of",
                                     name="of1")
                    for half, cc_ in ((0, 0), (1, 1)):
                        nc.sync.dma_start(
                            out=of0[:, :, half * CPC:(half + 1) * CPC],
                            in_=a2a_out[cc_][:, :, :].transpose([1, 0, 2]))
                    nc.sync.dma_start(
                        out=of1[:, :, 0:CPC],
                        in_=a2a_out[2][:, :, :].transpose([1, 0, 2]))
                nc.gpsimd.collective_compute(
                    "AllToAll", mybir.AluOpType.bypass,
                    replica_groups=[list(range(NCORES))],
                    ins=[a2a_in[c][:, :, :]],
                    outs=[a2a_out[c][:, :, :]],
                )

            # out-projection tail: only chunk 3's slice still needs loading
            # (after its A2A); pair 0's matmuls overlap that A2A's flight.
            nc.sync.dma_start(
                out=of1[:, :, CPC:2 * CPC],
                in_=a2a_out[3][:, :, :].transpose([1, 0, 2]))

            def outproj(pair, of):
                for nn in range(2):
                    zp = zp_ps.tile([P, 512], F32, tag="zp")
                    for j in range(NCORES):
                        nc.tensor.matmul(zp, of[:, j, :],
                                         wo_sb[:, j, nn * 512:(nn + 1) * 512],
                                         start=(j == 0), stop=(j == NCORES - 1))
                    zs = p2out.tile([P, 512], F32, tag="zs")
                    nc.vector.tensor_add(zs, zp,
                                         bo_b[:, nn * 512:(nn + 1) * 512])
                    nc.sync.dma_start(out=out_ext[pair * P:(pair + 1) * P,
                                                  nn * 512:(nn + 1) * 512],
                                      in_=zs)

            outproj(0, of0)
            outproj(1, of1)
    return nc


def _bf16(a):
    return np.ascontiguousarray(a).astype(ml_dtypes.bfloat16)


def _rope_tables(npos, pos0, g_first, g_second, n_first):
    """Tables [128, npos] for transposed-layout rope with g folded in.

    Row j (within a head, duplicated for 2 local heads):
      out[j] = t[j]*C[j] + t[sigma(j)]*S[j]
      j <  32: C[j]=g[j]*cos[n,j],     S[j]=-g[j+32]*sin[n,j]
      j >= 32: C[j]=g[j]*cos[n,j-32],  S[j]=+g[j-32]*sin[n,j-32]
    g switches from g_first to g_second at position n_first.
    """
    inv = 1.0 / (ROPE_BASE ** (np.arange(0, DH, 2, dtype=np.float64) / DH))
    pos = np.arange(pos0, pos0 + npos, dtype=np.float64)
    ang = pos[:, None] * inv[None, :]          # [npos, 32]
    cos = np.cos(ang).T                         # [32, npos]
    sin = np.sin(ang).T
    C = np.zeros((DH, npos), np.float64)
    S = np.zeros((DH, npos), np.float64)
    g = np.zeros((DH, npos), np.float64)
    g[:, :n_first] = np.asarray(g_first, np.float64)[:, None]
    if n_first < npos:
        g[:, n_first:] = np.asarray(g_second, np.float64)[:, None]
    C[:32] = cos
    C[32:] = cos
    C *= g
    S[:32] = -sin
    S[32:] = sin
    Srot = np.concatenate([g[32:], g[:32]], axis=0)  # g[sigma(j)]
    S *= Srot
    C2 = np.concatenate([C, C], axis=0)  # duplicate for 2 local heads
    S2 = np.concatenate([S, S], axis=0)
    return _bf16(C2), _bf16(S2)


_NC_CACHE = None


def kernel(x, y, W_qkv, W_ckv, W_out, b_out, g_q, g_k, g_ck, n_heads):
    global LAST_RESULT, _NC_CACHE
    x = np.asarray(x, np.float32)
    y = np.asarray(y, np.float32)
    W_qkv = np.asarray(W_qkv, np.float32)
    W_ckv = np.asarray(W_ckv, np.float32)
    W_out = np.asarray(W_out, np.float32)
    b_out = np.asarray(b_out, np.float32)

    def _prearr_x(a):
        # a [2048 pos, 1024 feat] -> [p, chunk, f, 512]:
        # element (f*128+p, c*512+ns) lands at [p, c, f, ns]
        return _bf16(a.T.reshape(8, P, 4, 512).transpose(1, 2, 0, 3))

    xT = _prearr_x(x[0])
    yT = _prearr_x(y[0])
    Wq, Wk, Wv = (W_qkv[:, i * D:(i + 1) * D] for i in range(3))
    Wck, Wcv = (W_ckv[:, i * D:(i + 1) * D] for i in range(2))

    def _prearr(w):
        # [1024, C] row f*128+p -> [p, f, c]: contiguous per-partition DMAs
        return _bf16(w.reshape(8, P, -1).transpose(1, 0, 2))

    woh = _prearr(W_out)
    boh = _bf16(b_out[None, :])

    cqh, sqh = _rope_tables(N, 0, g_q, g_q, N)
    ckch, cksh = _rope_tables(NK, 0, g_k, g_ck, N)
    hm = np.zeros((P, HL), np.float32)
    for h in range(HL):
        hm[h * DH:(h + 1) * DH, h] = 1.0 / DH
    hmh = _bf16(hm)
    hs = np.zeros((HL, P), np.float32)
    for h in range(HL):
        hs[h, h * DH:(h + 1) * DH] = 1.0
    hsh = _bf16(hs)
    idh = _bf16(np.eye(P, dtype=np.float32))
    # rotate-half permutation as a matmul: rot(t)[m] = t[sigma(m)],
    # sigma = swap 32-halves within each 64-dim head; rotm[k, m] = (k==sigma(m))
    rm = np.zeros((P, P), np.float32)
    for h in range(HL):
        b = h * DH
        for m in range(32):
            rm[b + 32 + m, b + m] = 1.0
            rm[b + m, b + 32 + m] = 1.0
    rmh = _bf16(rm)

    in_maps = []
    for c in range(NCORES):
        sl = slice(c * DL, (c + 1) * DL)
        in_maps.append({
            "xT": xT, "yT": yT,
            "wq": _prearr(Wq[:, sl]), "wk": _prearr(Wk[:, sl]),
            "wv": _prearr(Wv[:, sl]), "wck": _prearr(Wck[:, sl]),
            "wcv": _prearr(Wcv[:, sl]),
            "wo": woh, "bo": boh,
            "cq": cqh, "sq": sqh, "ckc": ckch, "cks": cksh,
            "hmask": hmh, "hsel": hsh, "ident": idh, "rotm": rmh,
        })

    if _NC_CACHE is None:
        _NC_CACHE = build_nc()
        if not _NC_CACHE.is_finalized():
            _NC_CACHE.finalize()
    nc = _NC_CACHE

    res = run_bass_kernel_spmd(
        nc, in_maps, core_ids=list(range(NCORES)),
        trace=bool(os.environ.get("BASS_TRACE")),
    )
    LAST_RESULT = res
    # out_ext rows on core j: pair*128 + half*64 + cc
    #   <-> global position (pair*2 + half)*512 + j*64 + cc
    out = np.empty((N, D), np.float32)
    for j in range(NCORES):
        o = np.asarray(res.results[j]["out"], np.float32)
        for ch in range(NCH):
            out[ch * CQ + j * CPC:ch * CQ + (j + 1) * CPC] = \
                o[ch * CPC:(ch + 1) * CPC]
    return out[None, :, :]


# revision 24
# speedup vs baseline: 1.5823x; 1.0388x over previous
"""Original staged baseline kernel (reconstructed) for re-benchmarking."""

import os

import numpy as np
import ml_dtypes

import concourse.bass as bass
import concourse.tile as tile
from concourse import bacc, mybir
from concourse.bass_utils import run_bass_kernel_spmd

BF16 = mybir.dt.bfloat16
F32 = mybir.dt.float32
I16 = mybir.dt.int16

N = 2048        # query positions
M = 2048        # cross positions
NK = N + M      # total keys
D = 1024        # model dim
H = 16          # heads
DH = 64         # head dim
HL = 2          # heads per core
DL = HL * DH    # local head dims = 128
F = 1024        # input features
P = 128
NCORES = 8
EPS = 1e-6
ROPE_BASE = 10000.0
SCALE = 0.125   # 1/sqrt(64)

EXA = SCALE * 128.0 * 1.4426950408889634
EXB = 16251.0

LAST_RESULT = None  # test harness reads exec_time_ns from here


def build_nc():
    nc = bacc.Bacc()

    # ---------------- DRAM parameters ----------------
    xT = nc.declare_dram_parameter("xT", [P, 4, 8, 512], BF16, isOutput=False)
    yT = nc.declare_dram_parameter("yT", [P, 4, 8, 512], BF16, isOutput=False)
    wq = nc.declare_dram_parameter("wq", [P, 8, DL], BF16, isOutput=False)
    wk = nc.declare_dram_parameter("wk", [P, 8, DL], BF16, isOutput=False)
    wv = nc.declare_dram_parameter("wv", [P, 8, DL], BF16, isOutput=False)
    wck = nc.declare_dram_parameter("wck", [P, 8, DL], BF16, isOutput=False)
    wcv = nc.declare_dram_parameter("wcv", [P, 8, DL], BF16, isOutput=False)
    wo = nc.declare_dram_parameter("wo", [P, 8, D], BF16, isOutput=False)
    bo = nc.declare_dram_parameter("bo", [1, D], BF16, isOutput=False)
    cq = nc.declare_dram_parameter("cq", [P, N], BF16, isOutput=False)
    sq = nc.declare_dram_parameter("sq", [P, N], BF16, isOutput=False)
    ckc = nc.declare_dram_parameter("ckc", [P, NK], BF16, isOutput=False)
    cks = nc.declare_dram_parameter("cks", [P, NK], BF16, isOutput=False)
    hmask = nc.declare_dram_parameter("hmask", [P, HL], BF16, isOutput=False)
    hsel = nc.declare_dram_parameter("hsel", [HL, P], BF16, isOutput=False)
    ident = nc.declare_dram_parameter("ident", [P, P], BF16, isOutput=False)
    out_ext = nc.declare_dram_parameter("out", [N // NCORES, D], F32, isOutput=True)

    # A2A bounce buffers (collectives can't touch I/O tensors). Separate
    # tensors per half so qh0's out-proj loads don't serialize on qh1's A2A.
    a2a_in = [nc.dram_tensor(f"a2a_in{i}", [NCORES, P, P], BF16)
              for i in range(2)]
    a2a_out = [nc.dram_tensor(f"a2a_out{i}", [NCORES, P, P], BF16)
               for i in range(2)]

    with tile.TileContext(nc) as tc, \
            tc.tile_pool(name="singles", bufs=1) as singles:

        # ---------------- static SBUF loads ----------------
        def load_w(param):
            t = singles.tile([P, 8, DL], BF16, tag=param.name + "_sb")
            nc.sync.dma_start(out=t, in_=param[:, :, :])
            return t

        # wq first so the first projection's weights are in flight before
        # anything else touches the DMA-issue queue.
        wq_sb = load_w(wq)

        # ---------------- phase 1 ----------------
        with tc.tile_pool(name="proj_ps", bufs=2, space="PSUM") as proj_ps, \
                tc.tile_pool(name="ssq_ps", bufs=2, space="PSUM") as ssq_ps, \
                tc.tile_pool(name="trps", bufs=2, space="PSUM") as trps, \
                tc.tile_pool(name="rsb_ps", bufs=2, space="PSUM") as rsb_ps, \
                tc.tile_pool(name="p1big", bufs=1) as p1big, \
                tc.tile_pool(name="rope", bufs=1) as rope, \
                tc.tile_pool(name="p1work", bufs=4) as p1work:

            # x chunk 0 issued before the remaining small loads: the first
            # projection starts ~7us earlier than with the bulk-load order.
            xT_sb = p1big.tile([P, 4, 8, 512], BF16)
            yT_sb = p1big.tile([P, 4, 8, 512], BF16)
            nc.sync.dma_start(out=xT_sb[:, 0], in_=xT[:, 0])
            wk_sb = load_w(wk)
            hmask_sb = singles.tile([P, HL], BF16)
            nc.sync.dma_start(out=hmask_sb, in_=hmask[:, :])
            nc.sync.dma_start(out=xT_sb[:, 1], in_=xT[:, 1])
            wv_sb = load_w(wv)
            for c4 in range(2, 4):
                nc.sync.dma_start(out=xT_sb[:, c4], in_=xT[:, c4])
            wck_sb = load_w(wck)
            wcv_sb = load_w(wcv)
            hsel_sb = singles.tile([HL, P], BF16)
            nc.sync.dma_start(out=hsel_sb, in_=hsel[:, :])
            ident_sb = singles.tile([P, P], BF16)
            nc.sync.dma_start(out=ident_sb, in_=ident[:, :])
            bo_sb = singles.tile([1, D], BF16)
            nc.sync.dma_start(out=bo_sb, in_=bo[0:1, :])
            bo_b = singles.tile([P, D], BF16)
            nc.gpsimd.partition_broadcast(bo_b[0:P, :], bo_sb[0:1, :],
                                          channels=P)
            eps2 = singles.tile([HL, 1], F32)
            nc.vector.memset(eps2, EPS)
            qTn = singles.tile([P, N], BF16)
            kTn = singles.tile([P, NK], BF16)
            v_all = singles.tile([P, NK // P, 130], BF16)
            nc.gpsimd.memset(v_all, 1.0)
            for c4 in range(4):
                nc.sync.dma_start(out=yT_sb[:, c4], in_=yT[:, c4])
            # rope tables after x/y: they're only needed ~60us in
            cq_sb = p1big.tile([P, N], BF16)
            sq_sb = p1big.tile([P, N], BF16)
            nc.sync.dma_start(out=cq_sb, in_=cq[:, :])
            nc.sync.dma_start(out=sq_sb, in_=sq[:, :])
            ckc_sb = p1big.tile([P, NK], BF16)
            cks_sb = p1big.tile([P, NK], BF16)
            nc.sync.dma_start(out=ckc_sb, in_=ckc[:, :])
            nc.sync.dma_start(out=cks_sb, in_=cks[:, :])

            ssq_all = p1big.tile([HL, 3 * N], F32)

            def proj_chunks(w_sb, src_sb, dst, dst_off, ssq_off):
                qsqs = []
                for t in range(4):
                    ps = proj_ps.tile([P, 512], F32, tag="proj")
                    for f in range(8):
                        nc.tensor.matmul(ps, w_sb[:, f, :],
                                         src_sb[:, t, f, :],
                                         start=(f == 0), stop=(f == 7))
                    raw = dst[:, dst_off + t * 512:dst_off + (t + 1) * 512]
                    nc.scalar.activation(
                        out=raw, in_=ps,
                        func=mybir.ActivationFunctionType.Copy)
                    qsq = p1work.tile([P, 512], BF16, tag="qsq")
                    nc.vector.tensor_mul(qsq, raw, raw)
                    qsqs.append(qsq)
                for t in range(4):
                    ssq = ssq_ps.tile([HL, 512], F32, tag="ssq")
                    nc.tensor.matmul(ssq, hmask_sb, qsqs[t],
                                     start=True, stop=True)
                    nc.vector.tensor_copy(
                        ssq_all[:, ssq_off + t * 512:ssq_off + (t + 1) * 512],
                        ssq)

            def rope_apply(dst, dst_off, ssq_off, c_sb, s_sb, tab_off):
                sl = slice(dst_off, dst_off + N)
                tab = slice(tab_off, tab_off + N)
                rsb_sb = rope.tile([P, N], BF16, name="rsb", tag="rsb")
                for t in range(4):
                    cs = slice(t * 512, (t + 1) * 512)
                    rp = rsb_ps.tile([P, 512], F32, tag="rsb_ps")
                    nc.tensor.matmul(
                        rp, hsel_sb,
                        rs_all[:, ssq_off + t * 512:ssq_off + (t + 1) * 512],
                        start=True, stop=True)
                    nc.scalar.activation(
                        out=rsb_sb[:, cs], in_=rp,
                        func=mybir.ActivationFunctionType.Copy)
                m1 = rope.tile([P, N], BF16, name="m1", tag="m1")
                nc.vector.tensor_mul(m1, dst[:, sl], c_sb[:, tab])
                t1r = rope.tile([P, N], BF16, name="t1r", tag="t1r")
                for h in range(HL):
                    b = h * DH
                    nc.sync.dma_start(out=t1r[b:b + 32, :],
                                      in_=dst[b + 32:b + 64, sl])
                    nc.sync.dma_start(out=t1r[b + 32:b + 64, :],
                                      in_=dst[b:b + 32, sl])
                r1 = rope.tile([P, N], BF16, name="r1", tag="r1")
                nc.vector.tensor_mul(r1, t1r, s_sb[:, tab])
                s2 = rope.tile([P, N], BF16, name="s2", tag="t1r")
                nc.vector.tensor_add(s2, m1, r1)
                nc.vector.tensor_mul(dst[:, sl], s2, rsb_sb)

            def vproj_chunks(lo, hi):
                for t in range(lo, hi):
                    src_sb, w_sb = (xT_sb, wv_sb) if t < 4 else (yT_sb, wcv_sb)
                    tt = t % 4
                    ps = proj_ps.tile([P, 512], F32, tag="proj")
                    for f in range(8):
                        nc.tensor.matmul(ps, w_sb[:, f, :],
                                         src_sb[:, tt, f, :],
                                         start=(f == 0), stop=(f == 7))
                    cs = slice(t * 512, (t + 1) * 512)
                    nc.vector.tensor_copy(vT_sb[:, cs], ps)

            vT_sb = p1big.tile([P, NK], BF16)
            proj_chunks(wq_sb, xT_sb, qTn, 0, 0)
            proj_chunks(wk_sb, xT_sb, kTn, 0, N)
            proj_chunks(wck_sb, yT_sb, kTn, N, 2 * N)
            vproj_chunks(0, 8)
            nc.scalar.activation(out=ssq_all, in_=ssq_all,
                                 func=mybir.ActivationFunctionType.Ln,
                                 bias=eps2)
            rs_all = p1big.tile([HL, 3 * N], BF16)
            nc.scalar.activation(out=rs_all, in_=ssq_all,
                                 func=mybir.ActivationFunctionType.Exp,
                                 scale=-0.5)

            for g in range(NK // 512):
                trp = trps.tile([P, 4, P], BF16, tag="trp")
                for i in range(4):
                    nc.tensor.transpose(trp[:, i, :],
                                        vT_sb[:, (4 * g + i) * P:(4 * g + i + 1) * P],
                                        ident_sb)
                sl4 = slice(4 * g, 4 * g + 4)
                nc.vector.tensor_copy(v_all[:, sl4, 0:64], trp[:, :, 0:64])
                nc.vector.tensor_copy(v_all[:, sl4, 65:129], trp[:, :, 64:128])

            rope_apply(qTn, 0, 0, cq_sb, sq_sb, 0)
            rope_apply(kTn, 0, N, ckc_sb, cks_sb, 0)
            rope_apply(kTn, N, 2 * N, ckc_sb, cks_sb, N)

        # ---------------- phase 2 ----------------
        with tc.tile_pool(name="st_ps", bufs=2, space="PSUM") as st_ps, \
                tc.tile_pool(name="pv_ps", bufs=1, space="PSUM") as pv_ps, \
                tc.tile_pool(name="p2work", bufs=3) as p2work, \
                tc.tile_pool(name="p2small", bufs=2) as p2small, \
                tc.tile_pool(name="p2out", bufs=2) as p2out:
            wo_sb = p2out.tile([P, 8, D], BF16, tag="wo_sb", bufs=1)
            nc.sync.dma_start(out=wo_sb, in_=wo[:, :, :])

            def of_load(qh):
                of_sb = p2out.tile([P, NCORES, P], BF16, name=f"of_sb{qh}",
                                   tag="of")
                for j in range(NCORES):
                    nc.sync.dma_start(out=of_sb[:, j, :],
                                      in_=a2a_out[qh][j, :, :])
                return of_sb

            def outproj(qh, of_sb):
                for nn in range(2):
                    zp = st_ps.tile([P, 512], F32, name="zp", tag="st",
                                    padded_shape=[P, 1024])
                    for j in range(NCORES):
                        nc.tensor.matmul(zp, of_sb[:, j, :],
                                         wo_sb[:, j, nn * 512:(nn + 1) * 512],
                                         start=(j == 0), stop=(j == NCORES - 1))
                    zs = p2out.tile([P, 512], F32, tag="zs")
                    nc.vector.tensor_add(zs, zp,
                                         bo_b[:, nn * 512:(nn + 1) * 512])
                    nc.sync.dma_start(out=out_ext[qh * P:(qh + 1) * P,
                                                  nn * 512:(nn + 1) * 512],
                                      in_=zs)

            of0 = None
            for qh in range(2):
                if qh == 1:
                    # qh0's out-proj input: issued here so the DMAs execute
                    # right after qh0's A2A lands, well BEFORE qh1's A2A is
                    # in flight (DMAs stall while a collective runs).
                    of0 = of_load(0)
                qsl = slice(qh * 1024, (qh + 1) * 1024)
                oT = p2work.tile([P, 1024], BF16, name=f"oT{qh}", tag="oT",
                                 bufs=2)
                pv = [pv_ps.tile([65, 1024], F32, name=f"pv{h}", tag=f"pv{h}",
                                 padded_shape=[P, 1024])
                      for h in range(HL)]
                def emit_pv(kc, es):
                    for h in range(HL):
                        for c in range(2):
                            nc.tensor.matmul(
                                pv[h][:, c * 512:(c + 1) * 512],
                                v_all[:, kc, h * 65:(h + 1) * 65],
                                es[h][:, c * 512:(c + 1) * 512],
                                start=(kc == 0), stop=(kc == NK // P - 1))

                es_prev = None
                for kc in range(NK // P):
                    sts = [st_ps.tile([P, 1024], F32, name="st", tag="st")
                           for _ in range(HL)]
                    for c in range(2):
                        for h in range(HL):
                            hs = slice(h * DH, (h + 1) * DH)
                            nc.tensor.matmul(
                                sts[h][:, c * 512:(c + 1) * 512],
                                kTn[hs, kc * P:(kc + 1) * P],
                                qTn[hs, qh * 1024 + c * 512: qh * 1024 + (c + 1) * 512],
                                start=True, stop=True)
                    if es_prev is not None:
                        emit_pv(kc - 1, es_prev)
                    es = []
                    for h in range(HL):
                        e = p2work.tile([P, 1024], BF16, name="es", tag="es",
                                        bufs=6)
                        if h == 0:
                            nc.scalar.activation(
                                out=e, in_=sts[h],
                                func=mybir.ActivationFunctionType.Exp,
                                scale=SCALE)
                        else:
                            nc.vector.tensor_scalar(
                                out=e.bitcast(I16), in0=sts[h],
                                scalar1=EXA, scalar2=EXB,
                                op0=mybir.AluOpType.mult,
                                op1=mybir.AluOpType.add)
                        es.append(e)
                    es_prev = es
                emit_pv(NK // P - 1, es_prev)
                lnds = []
                for h in range(HL):
                    lnd = p2small.tile([1, 1024], F32, tag="lnd")
                    nc.scalar.activation(out=lnd, in_=pv[h][64:65, :],
                                         func=mybir.ActivationFunctionType.Ln)
                    lnds.append(lnd)
                for h in range(HL):
                    rdc = p2small.tile([1, 1024], BF16, tag="rdc")
                    nc.scalar.activation(out=rdc, in_=lnds[h],
                                         func=mybir.ActivationFunctionType.Exp,
                                         scale=-1.0)
                    rdb = p2small.tile([DH, 1024], BF16, tag="rdb")
                    nc.gpsimd.partition_broadcast(rdb[0:DH, :], rdc[0:1, :],
                                                  channels=DH)
                    nc.vector.tensor_mul(oT[h * DH:(h + 1) * DH, :],
                                         pv[h][0:64, :], rdb)
                nc.sync.dma_start(
                    out=a2a_in[qh][:, :, :].transpose([1, 0, 2]),
                    in_=oT[:, :])
                nc.gpsimd.collective_compute(
                    "AllToAll", mybir.AluOpType.bypass,
                    replica_groups=[list(range(NCORES))],
                    ins=[a2a_in[qh][:, :, :]],
                    outs=[a2a_out[qh][:, :, :]],
                )
            # pair 0's matmuls execute inside qh1's A2A flight window; of1's
            # DMAs (emitted after) unblock when that A2A completes.
            outproj(0, of0)
            of1 = of_load(1)
            outproj(1, of1)
    return nc


def _bf16(a):
    return np.ascontiguousarray(a).astype(ml_dtypes.bfloat16)


def _rope_tables(npos, pos0, g_first, g_second, n_first):
    inv = 1.0 / (ROPE_BASE ** (np.arange(0, DH, 2, dtype=np.float64) / DH))
    pos = np.arange(pos0, pos0 + npos, dtype=np.float64)
    ang = pos[:, None] * inv[None, :]
    cos = np.cos(ang).T
    sin = np.sin(ang).T
    C = np.zeros((DH, npos), np.float64)
    S = np.zeros((DH, npos), np.float64)
    g = np.zeros((DH, npos), np.float64)
    g[:, :n_first] = np.asarray(g_first, np.float64)[:, None]
    if n_first < npos:
        g[:, n_first:] = np.asarray(g_second, np.float64)[:, None]
    C[:32] = cos
    C[32:] = cos
    C *= g
    S[:32] = -sin
    S[32:] = sin
    Srot = np.concatenate([g[32:], g[:32]], axis=0)
    S *= Srot
    C2 = np.concatenate([C, C], axis=0)
    S2 = np.concatenate([S, S], axis=0)
    return _bf16(C2), _bf16(S2)


_NC_CACHE = None


def kernel(x, y, W_qkv, W_ckv, W_out, b_out, g_q, g_k, g_ck, n_heads):
    global LAST_RESULT, _NC_CACHE
    x = np.asarray(x, np.float32)
    y = np.asarray(y, np.float32)
    W_qkv = np.asarray(W_qkv, np.float32)
    W_ckv = np.asarray(W_ckv, np.float32)
    W_out = np.asarray(W_out, np.float32)
    b_out = np.asarray(b_out, np.float32)

    def _prearr_x(a):
        return _bf16(a.T.reshape(8, P, 4, 512).transpose(1, 2, 0, 3))

    xT = _prearr_x(x[0])
    yT = _prearr_x(y[0])
    Wq, Wk, Wv = (W_qkv[:, i * D:(i + 1) * D] for i in range(3))
    Wck, Wcv = (W_ckv[:, i * D:(i + 1) * D] for i in range(2))

    def _prearr(w):
        return _bf16(w.reshape(8, P, -1).transpose(1, 0, 2))

    woh = _prearr(W_out)
    boh = _bf16(b_out[None, :])

    cqh, sqh = _rope_tables(N, 0, g_q, g_q, N)
    ckch, cksh = _rope_tables(NK, 0, g_k, g_ck, N)
    hm = np.zeros((P, HL), np.float32)
    for h in range(HL):
        hm[h * DH:(h + 1) * DH, h] = 1.0 / DH
    hmh = _bf16(hm)
    hs = np.zeros((HL, P), np.float32)
    for h in range(HL):
        hs[h, h * DH:(h + 1) * DH] = 1.0
    hsh = _bf16(hs)
    idh = _bf16(np.eye(P, dtype=np.float32))

    in_maps = []
    for c in range(NCORES):
        sl = slice(c * DL, (c + 1) * DL)
        in_maps.append({
            "xT": xT, "yT": yT,
            "wq": _prearr(Wq[:, sl]), "wk": _prearr(Wk[:, sl]),
            "wv": _prearr(Wv[:, sl]), "wck": _prearr(Wck[:, sl]),
            "wcv": _prearr(Wcv[:, sl]),
            "wo": woh, "bo": boh,
            "cq": cqh, "sq": sqh, "ckc": ckch, "cks": cksh,
            "hmask": hmh, "hsel": hsh, "ident": idh,
        })

    if _NC_CACHE is None:
        _NC_CACHE = build_nc()
        if not _NC_CACHE.is_finalized():
            _NC_CACHE.finalize()
    nc = _NC_CACHE

    res = run_bass_kernel_spmd(
        nc, in_maps, core_ids=list(range(NCORES)),
        trace=bool(os.environ.get("BASS_TRACE")),
    )
    LAST_RESULT = res
    out = np.empty((N, D), np.float32)
    for c in range(NCORES):
        o = np.asarray(res.results[c]["out"], np.float32)
        out[c * P:(c + 1) * P] = o[0:P]
        out[N // 2 + c * P:N // 2 + (c + 1) * P] = o[P:2 * P]
    return out[None, :, :]
